# revision 18
# baseline (speedup 1.0000x reference)
"""Trainium2 Bass kernel for nn_CellPerformer (FAVOR+ performer, B=4 N=2048 D=128 H=8 L=4).

Sharding: 8 cores = 4 batches x 2 head-groups (4 heads each). Each core
holds the full residual stream for its batch; attention is head-parallel
within a (batch) pair of cores; the per-layer out-projection partial sums
are AllReduce'd across the pair (chunk-pipelined, overlapped with the
attention epilogue + FFN); LN/FFN are computed redundantly (cheap).

Math plan (validated in numpy vs fp64 reference, relerr ~1.6e-4 from the
structural approximations + ~3e-3 from bf16):
 - KEPS (+1e-4 inside the FAVOR feature) dropped entirely
 - FAVOR max-subtraction dropped (mx=0)
 - q-side diag exp(-|q|^2/2) cancels between numerator and denominator
   (attention output is 0-homogeneous in qf) -> not computed at all
 - k-side diag kept, fused as the ACT-exp per-partition bias
 - LN gamma/beta folded into the following projection weights on the host
 - all big matmuls bf16 with fp32 PSUM accumulation; enc/dec fp32
 - gelu = tanh approximation (matches jax.nn.gelu default)
 - v-bias handled exactly via a rank-1 correction on ctx

Scheduling: ACT table loads minimized by clustering Ln ops (Square/Copy/
Identity/Exp co-reside in one table set; Ln and Gelu force switches, so
they are batched 4-wide).  Next layer's LN1 + q/k/v projections are
emitted in the current layer's epilogue so the PE stays busy during the
ACT-heavy LN/gelu stretch.  PSUM: psK 2x[128,1024] (kf px double-buffer /
merged qfe pairs / FFN pf), psC 1x[128,640] (pctx / potq / pf2), psA
2x[128,512] (proj, pa, rows) + 4x[128,512] (LN stats).
"""
import math
import os
import sys

import numpy as np
import ml_dtypes

for _p in ('/opt/trn_rl_repo', '/root/.axon_site/_ro/trn_rl_repo'):
    if os.path.isdir(_p) and _p not in sys.path:
        sys.path.append(_p)

import concourse.bass as bass
import concourse.tile as tile
from concourse import bacc, mybir
from concourse.bass_utils import run_bass_kernel_spmd
from concourse.masks import make_identity

FP = mybir.dt.float32
BF = mybir.dt.bfloat16
AF = mybir.ActivationFunctionType
ALU = mybir.AluOpType

B, NTOK, M = 4, 2048, 35
D, H, L = 128, 8, 4
FD = 621
FF = 512
HPC = 4                      # heads per core
NORM = D ** -0.25
FC = [128, 128, 128, 128, 109]   # FD chunk sizes
NCH = 5
CB_PER_LAYER = 14            # colbias columns per layer
GELU_AF = None               # resolved at build: AF.Gelu_apprx_tanh


def build_program(n_tokens=NTOK, n_cores=8, zero_qkb=True):
    NT = n_tokens // 128      # 128-token tiles
    NQ = n_tokens // 512      # 512-token chunks
    groups = [[2 * i, 2 * i + 1] for i in range(n_cores // 2)]

    nc = bacc.Bacc("TRN2", target_bir_lowering=False, debug=False,
                   num_devices=n_cores)

    xT = nc.dram_tensor("xT", [M, n_tokens], FP, kind="ExternalInput")
    encw = nc.dram_tensor("encw", [M, D], FP, kind="ExternalInput")
    decw = nc.dram_tensor("decw", [D, 1], FP, kind="ExternalInput")
    wqkv = nc.dram_tensor("wqkv", [L, 3, D, 512], BF, kind="ExternalInput")
    pm = nc.dram_tensor("pm", [L, D, FD], BF, kind="ExternalInput")
    wo = nc.dram_tensor("wo", [L, D, HPC * D], BF, kind="ExternalInput")
    w1 = nc.dram_tensor("w1", [L, D, FF], BF, kind="ExternalInput")
    w2 = nc.dram_tensor("w2", [L, FF // 4, 4 * D], BF, kind="ExternalInput")
    colbias = nc.dram_tensor("colbias", [D, 2 + CB_PER_LAYER * L], FP,
                             kind="ExternalInput")
    bvrow = nc.dram_tensor("bvrow", [1, HPC * L * D], BF, kind="ExternalInput")
    out = nc.dram_tensor("out", [1, n_tokens], FP, kind="ExternalOutput")

    from contextlib import ExitStack
    with tile.TileContext(nc) as tc:
        with ExitStack() as st:
            _emit(st, tc, nc, locals(), NT, NQ, n_tokens, groups, zero_qkb)

    nc.compile()
    return nc


def _emit(st, tc, nc, IO, NT, NQ, n_tokens, groups, zero_qkb):
    xT, encw, decw = IO['xT'], IO['encw'], IO['decw']
    wqkv, pm, wo, w1, w2 = IO['wqkv'], IO['pm'], IO['wo'], IO['w1'], IO['w2']
    colbias_d, bvrow_d, out_d = IO['colbias'], IO['bvrow'], IO['out']

    consts = st.enter_context(tc.tile_pool(name="consts", bufs=1))
    wts = st.enter_context(tc.tile_pool(name="wts", bufs=2))
    acts = st.enter_context(tc.tile_pool(name="acts", bufs=1))
    sm = st.enter_context(tc.tile_pool(name="sm", bufs=2))
    psK = st.enter_context(tc.tile_pool(name="psK", bufs=2, space="PSUM"))
    psC = st.enter_context(tc.tile_pool(name="psC", bufs=1, space="PSUM"))
    psA = st.enter_context(tc.tile_pool(name="psA", bufs=2, space="PSUM"))
    dram = st.enter_context(tc.tile_pool(name="dram", bufs=8, space="DRAM"))

    # ---- constants ----
    ones_bf = consts.tile([128, 1], BF, tag="ones_bf")
    nc.vector.memset(ones_bf, 1.0)
    onesc = consts.tile([128, 128], FP, tag="onesc")
    nc.vector.memset(onesc, 1.0 / 128.0)
    onesc_bf = consts.tile([128, 128], BF, tag="onesc_bf")
    nc.vector.memset(onesc_bf, 1.0 / 128.0)
    halfneg = consts.tile([128, 1], BF, tag="halfneg")
    nc.vector.memset(halfneg, -0.5)
    ident_bf = consts.tile([128, 128], BF, tag="ident_bf")
    make_identity(nc, ident_bf)
    eps_col = consts.tile([128, 1], FP, tag="eps_col")
    nc.vector.memset(eps_col, 1e-5)
    cb = consts.tile([128, 2 + CB_PER_LAYER * L], FP, tag="cb")
    nc.sync.dma_start(out=cb, in_=colbias_d[:, :])
    bvr = consts.tile([1, HPC * L * D], BF, tag="bvr")
    nc.sync.dma_start(out=bvr, in_=bvrow_d[:, :])
    encw_t = consts.tile([M, 128], FP, tag="encw_t")
    nc.sync.dma_start(out=encw_t, in_=encw[:, :])
    decw_t = consts.tile([128, 1], FP, tag="decw_t")
    nc.sync.dma_start(out=decw_t, in_=decw[:, :])
    xT_t = consts.tile([M, n_tokens], FP, tag="xT_t")
    nc.sync.dma_start(out=xT_t, in_=xT[:, :])

    y = acts.tile([128, n_tokens], FP, tag="y")

    # ---- encoder (fp32): y^T = encw^T @ x^T + enc_b ----
    for tq in range(NQ):
        p = psA.tile([128, 512], FP, tag="u")
        nc.tensor.matmul(p, encw_t, xT_t[:, tq * 512:(tq + 1) * 512],
                         start=True, stop=True)
        nc.vector.tensor_scalar_add(y[:, tq * 512:(tq + 1) * 512], p, cb[:, 0:1])

    def emit_ln(dst):
        """dst = (y - mu)*rsqrt(var+eps).  Ln and Exp ops batched 4-wide so
        the ACT table set switches at most twice.  ymu is computed eagerly
        so the PSUM stat tiles are released before the batched Ln phase."""
        ymus, work = [], []
        for tq in range(NQ):
            sl = slice(tq * 512, (tq + 1) * 512)
            ysq = sm.tile([128, 512], BF, tag="ysq", bufs=2)
            nc.vector.tensor_mul(ysq, y[:, sl], y[:, sl])
            pS = psA.tile([128, 512], FP, tag="u")
            nc.tensor.matmul(pS, onesc, y[:, sl], start=True, stop=True)
            pSS = psA.tile([128, 512], FP, tag="u")
            nc.tensor.matmul(pSS, onesc_bf, ysq, start=True, stop=True)
            musq = sm.tile([128, 512], FP, tag="musq", bufs=1)
            nc.scalar.activation(musq, pS, AF.Square)
            ymu = sm.tile([128, 512], BF, tag="ymu", bufs=4)
            nc.vector.tensor_sub(ymu, y[:, sl], pS)
            varr = sm.tile([128, 512], FP, tag="varr", bufs=4)
            nc.vector.tensor_sub(varr, pSS, musq)
            ymus.append(ymu)
            work.append(varr)
        for tq in range(NQ):
            lnv = sm.tile([128, 512], FP, tag="lnv", bufs=4)
            nc.scalar.activation(lnv, work[tq], AF.Ln, bias=eps_col[:, 0:1])
            work[tq] = lnv
        for tq in range(NQ):
            rs = sm.tile([128, 512], BF, tag="rs", bufs=4)
            nc.scalar.activation(rs, work[tq], AF.Exp, scale=-0.5)
            work[tq] = rs
        for tq in range(NQ):
            sl = slice(tq * 512, (tq + 1) * 512)
            nc.vector.tensor_mul(dst[:, sl], ymus[tq], work[tq])

    def emit_proj(li, h, qh, kh, vK, ndk):
        """q/k/v projections for all heads + k-diag columns."""
        base = 2 + CB_PER_LAYER * li
        wqkv_t = wts.tile([128, 3, 512], BF, tag="wqkv_t", name="wqkv_t")
        nc.sync.dma_start(out=wqkv_t, in_=wqkv[li].rearrange("t d f -> d t f"))
        for h4 in range(HPC):
            wq_c = wqkv_t[:, 0, h4 * 128:(h4 + 1) * 128]
            wk_c = wqkv_t[:, 1, h4 * 128:(h4 + 1) * 128]
            for tq in range(NQ):
                sl = slice(tq * 512, (tq + 1) * 512)
                pq = psA.tile([128, 512], FP, tag="u")
                nc.tensor.matmul(pq, wq_c, h[:, sl], start=True, stop=True)
                if zero_qkb:
                    nc.vector.tensor_copy(qh[h4][:, sl], pq)
                else:
                    nc.vector.tensor_scalar_add(
                        qh[h4][:, sl], pq, cb[:, base + h4:base + h4 + 1])
                pk = psA.tile([128, 512], FP, tag="u")
                nc.tensor.matmul(pk, wk_c, h[:, sl], start=True, stop=True)
                if zero_qkb:
                    nc.vector.tensor_copy(kh[h4][:, sl], pk)
                else:
                    nc.vector.tensor_scalar_add(
                        kh[h4][:, sl], pk, cb[:, base + 4 + h4:base + 5 + h4])
        for tt in range(NT):
            pv = psK.tile([128, 1024], FP, tag="px")
            nc.tensor.matmul(pv[:, 0:512], h[:, tt * 128:(tt + 1) * 128],
                             wqkv_t[:, 2, :], start=True, stop=True)
            nc.vector.tensor_copy(vK[:, tt * 512:(tt + 1) * 512], pv[:, 0:512])
        # k-diag columns: ndk[tok, h4*16+tt] = -0.5 * sum_d k^2
        pkc = psA.tile([128, 64], FP, tag="u")
        ksqs = []
        for h4 in range(HPC):
            ksq = sm.tile([128, n_tokens], BF, tag="ksq", bufs=2, name="ksq")
            nc.vector.tensor_mul(ksq, kh[h4], kh[h4])
            ksqs.append(ksq)
        for h4 in range(HPC):
            for tt in range(NT):
                nc.tensor.matmul(pkc[:, h4 * 16 + tt:h4 * 16 + tt + 1],
                                 ksqs[h4][:, tt * 128:(tt + 1) * 128], halfneg,
                                 start=(h4 == 0 and tt == 0), stop=True)
        nc.vector.tensor_copy(ndk, pkc)

    # ---- layer-0 prologue ----
    h = acts.tile([128, n_tokens], BF, tag="h", bufs=2)
    emit_ln(h)
    qh = [acts.tile([128, n_tokens], BF, tag=f"qh{h4}", name=f"qh{h4}", bufs=1)
          for h4 in range(HPC)]
    kh = [acts.tile([128, n_tokens], BF, tag=f"kh{h4}", name=f"kh{h4}", bufs=1)
          for h4 in range(HPC)]
    vK = acts.tile([128, NT * 512], BF, tag="vK", bufs=1)
    ndk = sm.tile([128, 64], FP, tag="ndk", bufs=2)
    emit_proj(0, h, qh, kh, vK, ndk)

    for li in range(L):
        base = 2 + CB_PER_LAYER * li
        # ---- stage attention/ffn weights ----
        pm_t = wts.tile([128, FD], BF, tag="pm_t")
        nc.sync.dma_start(out=pm_t, in_=pm[li])
        wo_t = wts.tile([128, HPC * 128], BF, tag="wo_t")
        nc.sync.dma_start(out=wo_t, in_=wo[li])
        w1_t = wts.tile([128, FF], BF, tag="w1_t")
        nc.sync.dma_start(out=w1_t, in_=w1[li])
        w2_t = wts.tile([128, 4 * 128], BF, tag="w2_t")
        nc.sync.dma_start(out=w2_t, in_=w2[li])

        # ---- per-head: kf features, ctx, ksum, Raug ----
        Raugs = []
        for h4 in range(HPC):
            pctx = psC.tile([128, 640], FP, tag="ctx")
            pksa = psA.tile([1, 512], FP, tag="u")
            pksb = psA.tile([1, 128], FP, tag="u")
            for tt in range(NT):
                px = psK.tile([128, 1024], FP, tag="px")
                kh_t = kh[h4][:, tt * 128:(tt + 1) * 128]
                nc.tensor.matmul(px[:, 0:512], kh_t, pm_t[:, 0:512],
                                 start=True, stop=True)
                nc.tensor.matmul(px[:, 512:FD], kh_t, pm_t[:, 512:FD],
                                 start=True, stop=True)
                kf = acts.tile([128, FD], BF, tag="kf", bufs=6, name="kf")
                nc.scalar.activation(kf, px[:, 0:FD], AF.Exp,
                                     bias=ndk[:, h4 * 16 + tt:h4 * 16 + tt + 1])
                vk_t = vK[:, tt * 512 + h4 * 128:tt * 512 + (h4 + 1) * 128]
                nc.tensor.matmul(pctx[:, 0:512], vk_t, kf[:, 0:512],
                                 start=(tt == 0), stop=False)
                nc.tensor.matmul(pctx[:, 512:FD], vk_t, kf[:, 512:FD],
                                 start=(tt == 0), stop=False)
                nc.tensor.matmul(pksa, ones_bf, kf[:, 0:512],
                                 start=(tt == 0), stop=(tt == NT - 1))
                nc.tensor.matmul(pksb[0:1, 0:FD - 512], ones_bf, kf[:, 512:FD],
                                 start=(tt == 0), stop=(tt == NT - 1))
            ks_row = sm.tile([1, FD], BF, tag="ks_row", bufs=2)
            nc.vector.tensor_copy(ks_row[0:1, 0:512], pksa)
            nc.vector.tensor_copy(ks_row[0:1, 512:FD], pksb[0:1, 0:FD - 512])
            # v-bias rank-1 correction: ctx += bv (x) ksum
            rb = bvr[0:1, (li * HPC + h4) * 128:(li * HPC + h4 + 1) * 128]
            nc.tensor.matmul(pctx[:, 0:512], rb, ks_row[0:1, 0:512],
                             start=False, stop=True)
            nc.tensor.matmul(pctx[:, 512:FD], rb, ks_row[0:1, 512:FD],
                             start=False, stop=True)
            ctxT = sm.tile([128, FD], BF, tag="ctxT", bufs=2)
            nc.vector.tensor_copy(ctxT, pctx[:, 0:FD])
            # ksum -> columns [FC, 1] per chunk (rank-1 matmuls, one bank)
            pst = psA.tile([128, 8], FP, tag="u")
            nc.vector.memset(pst, 0.0)
            for c in range(NCH):
                nc.tensor.matmul(pst[0:FC[c], c:c + 1],
                                 ks_row[0:1, c * 128:c * 128 + FC[c]],
                                 ones_bf[0:1, 0:1],
                                 start=(c == 0), stop=True)
            kscol = sm.tile([128, 8], BF, tag="kscol", bufs=2)
            nc.vector.tensor_copy(kscol, pst)
            # Raug chunks: [ctx^T chunk @ wo | ksum chunk]
            Raug = sm.tile([128, NCH * 129], BF, tag=f"Raug{h4}", bufs=1,
                           name=f"Raug{h4}")
            for c in range(NCH):
                pcw = psA.tile([128, 512], FP, tag="u")
                nc.tensor.matmul(pcw[0:FC[c], 0:128],
                                 ctxT[:, c * 128:c * 128 + FC[c]],
                                 wo_t[:, h4 * 128:(h4 + 1) * 128],
                                 start=True, stop=True)
                nc.vector.tensor_copy(Raug[0:FC[c], c * 129:c * 129 + 128],
                                      pcw[0:FC[c], 0:128])
            nc.vector.tensor_copy(
                Raug.rearrange("p (c w) -> p c w", w=129)[:, :, 128],
                kscol[:, 0:NCH])
            Raugs.append(Raug)

        # ---- pa phase: per 512-chunk: qfe gen (pair-merged exps), pa,
        #      transpose-accumulate into o^T PSUM, wire out, AllReduce.
        #      qfe generation runs one chunk ahead of pa so the PE never
        #      waits on the ACT exp stream. ----
        ccouts = [None] * NQ

        def emit_qfe_gen(tq):
            sl = slice(tq * 512, (tq + 1) * 512)
            qfes = []
            for h4 in range(HPC):
                qfe = sm.tile([128, NCH * 512], BF, tag=f"qfe{h4}", bufs=2,
                              name=f"qfe{h4}")
                for cp in range(3):      # chunk pairs (0,1), (2,3), (4,)
                    pqf = psK.tile([128, 1024], FP, tag="px")
                    w = 0
                    for c in range(2 * cp, min(2 * cp + 2, NCH)):
                        nc.tensor.matmul(pqf[0:FC[c], w:w + 512],
                                         pm_t[:, c * 128:c * 128 + FC[c]],
                                         qh[h4][:, sl], start=True, stop=True)
                        w += 512
                    nc.scalar.activation(
                        qfe[:, 2 * cp * 512:2 * cp * 512 + w],
                        pqf[:, 0:w], AF.Exp)
                qfes.append(qfe)
            return qfes

        def emit_pa_body(tq, qfes):
            potq = psC.tile([128, 640], FP, tag="ctx")
            for t4 in range(4):
                ots = []
                for h4 in range(HPC):
                    pa = psA.tile([128, 512], FP, tag="u")
                    for c in range(NCH):
                        nc.tensor.matmul(
                            pa[:, 0:129],
                            qfes[h4][0:FC[c], c * 512 + t4 * 128:
                                     c * 512 + (t4 + 1) * 128],
                            Raugs[h4][0:FC[c], c * 129:(c + 1) * 129],
                            start=(c == 0), stop=(c == NCH - 1))
                    sc = sm.tile([128, 1], FP, tag="sc", bufs=4)
                    nc.vector.reciprocal(sc, pa[:, 128:129])
                    ot = sm.tile([128, 128], BF, tag="ot", bufs=8)
                    nc.vector.tensor_scalar_mul(ot, pa[:, 0:128], sc)
                    ots.append(ot)
                for h4 in range(HPC):
                    nc.tensor.matmul(potq[:, t4 * 128:(t4 + 1) * 128],
                                     ots[h4], ident_bf,
                                     start=(t4 == 0 and h4 == 0),
                                     stop=(h4 == HPC - 1))
            wire = sm.tile([128, 512], BF, tag="wire", bufs=2)
            nc.scalar.activation(wire, potq[:, 0:512], AF.Identity,
                                 bias=cb[:, base + 12:base + 13])
            ccin = dram.tile([128, 512], BF, tag="ccin", bufs=8)
            ccout = dram.tile([128, 512], BF, tag="ccout", bufs=8)
            nc.sync.dma_start(out=ccin, in_=wire)
            nc.gpsimd.collective_compute(
                "AllReduce", ALU.add, replica_groups=groups,
                ins=[ccin.opt()], outs=[ccout.opt()])
            ccouts[tq] = ccout

        qprev = emit_qfe_gen(0)
        for tq in range(1, NQ):
            qnext = emit_qfe_gen(tq)
            emit_pa_body(tq - 1, qprev)
            qprev = qnext
        emit_pa_body(NQ - 1, qprev)

        # ---- epilogue: y += AR result, LN2 (Ln-batched), FFN; then next
        #      layer's LN1 + projections to keep the PE fed ----
        for tq in range(NQ):
            sl = slice(tq * 512, (tq + 1) * 512)
            asum = sm.tile([128, 512], BF, tag="asum", bufs=2)
            nc.sync.dma_start(out=asum, in_=ccouts[tq])
            nc.vector.tensor_add(y[:, sl], y[:, sl], asum)
        h2 = acts.tile([128, n_tokens], BF, tag="h", bufs=2, name="h2")
        emit_ln(h2)
        for tq in range(NQ):
            sl = slice(tq * 512, (tq + 1) * 512)
            gl = sm.tile([128, 4 * 512], BF, tag="gl", bufs=1)
            for c in range(4):
                pf = psK.tile([128, 1024], FP, tag="px")
                nc.tensor.matmul(pf[:, 0:512], w1_t[:, c * 128:(c + 1) * 128],
                                 h2[:, sl], start=True, stop=True)
                nc.scalar.activation(gl[:, c * 512:(c + 1) * 512],
                                     pf[:, 0:512], GELU_AF or AF.Gelu_apprx_tanh,
                                     bias=cb[:, base + 8 + c:base + 9 + c])
            pf2 = psC.tile([128, 640], FP, tag="ctx")
            for c in range(4):
                nc.tensor.matmul(pf2[:, 0:512], w2_t[:, c * 128:(c + 1) * 128],
                                 gl[:, c * 512:(c + 1) * 512],
                                 start=(c == 0), stop=(c == 3))
            ffnd = sm.tile([128, 512], BF, tag="ffnd", bufs=2)
            nc.scalar.activation(ffnd, pf2[:, 0:512], AF.Identity,
                                 bias=cb[:, base + 13:base + 14])
            nc.vector.tensor_add(y[:, sl], y[:, sl], ffnd)
        if li < L - 1:
            h = acts.tile([128, n_tokens], BF, tag="h", bufs=2, name="h")
            emit_ln(h)
            qh = [acts.tile([128, n_tokens], BF, tag=f"qh{h4}",
                            name=f"qh{h4}", bufs=1) for h4 in range(HPC)]
            kh = [acts.tile([128, n_tokens], BF, tag=f"kh{h4}",
                            name=f"kh{h4}", bufs=1) for h4 in range(HPC)]
            vK = acts.tile([128, NT * 512], BF, tag="vK", bufs=1)
            ndk = sm.tile([128, 64], FP, tag="ndk", bufs=2)
            emit_proj(li + 1, h, qh, kh, vK, ndk)

    # ---- decoder (fp32) ----
    for tq in range(NQ):
        pd = psA.tile([1, 512], FP, tag="u")
        nc.tensor.matmul(pd, decw_t, y[:, tq * 512:(tq + 1) * 512],
                         start=True, stop=True)
        orow = sm.tile([1, 512], FP, tag="orow", bufs=2)
        nc.vector.tensor_scalar_add(orow, pd, cb[0:1, 1:2])
        nc.sync.dma_start(out=out_d[0:1, tq * 512:(tq + 1) * 512], in_=orow)


# --------------------------------------------------------------------------
# host side
# --------------------------------------------------------------------------

def _bf(x):
    return np.ascontiguousarray(x).astype(ml_dtypes.bfloat16)


def _f32(x):
    return np.ascontiguousarray(x, dtype=np.float32)


def host_prep(inputs, n_tokens=NTOK, n_cores=8):
    """Full inputs -> per-core input dicts."""
    inp = {k: np.asarray(v, dtype=np.float32) for k, v in inputs.items()}
    maps = []
    for core in range(n_cores):
        b = core // 2
        hg = core % 2
        hsl = slice(hg * HPC * D, (hg + 1) * HPC * D)   # 512 head cols
        C = 2 + CB_PER_LAYER * L
        colbias = np.zeros((D, C), np.float32)
        colbias[:, 0] = inp['enc_b']
        colbias[0, 1] = inp['dec_b'][0]
        wqkv = np.zeros((L, 3, D, 512), np.float32)
        pmT = np.zeros((L, D, FD), np.float32)
        woA = np.zeros((L, D, HPC * D), np.float32)
        w1A = np.zeros((L, D, FF), np.float32)
        w2A = np.zeros((L, FF // 4, 4 * D), np.float32)
        bvrow = np.zeros((HPC * L, D), np.float32)
        for i in range(L):
            g1, b1 = inp['ln1_g'][i], inp['ln1_b'][i]
            g2, b2v = inp['ln2_g'][i], inp['ln2_b'][i]
            base = 2 + CB_PER_LAYER * i
            wqkv[i, 0] = g1[:, None] * inp['wq'][i][:, hsl] * NORM
            wqkv[i, 1] = g1[:, None] * inp['wk'][i][:, hsl] * NORM
            wqkv[i, 2] = g1[:, None] * inp['wv'][i][:, hsl]
            bq_eff = (b1 @ inp['wq'][i][:, hsl] + inp['bq'][i][hsl]) * NORM
            bk_eff = (b1 @ inp['wk'][i][:, hsl] + inp['bk'][i][hsl]) * NORM
            bv_eff = b1 @ inp['wv'][i][:, hsl] + inp['bv'][i][hsl]
            colbias[:, base:base + 4] = bq_eff.reshape(4, 128).T
            colbias[:, base + 4:base + 8] = bk_eff.reshape(4, 128).T
            pmT[i] = inp['proj'][i].T
            wo_sl = inp['wo'][i][hsl, :].reshape(HPC, D, D)      # [h, d, m]
            woA[i] = wo_sl.transpose(1, 0, 2).reshape(D, HPC * D)
            w1A[i] = g2[:, None] * inp['w1'][i]
            b1_eff = b2v @ inp['w1'][i] + inp['b1'][i]
            colbias[:, base + 8:base + 12] = b1_eff.reshape(4, 128).T
            w2c = inp['w2'][i].reshape(4, 128, D)
            w2A[i] = w2c.transpose(1, 0, 2).reshape(128, 4 * D)
            # bo halved: the pair AllReduce sums it from both cores
            colbias[:, base + 12] = 0.5 * inp['bo'][i]
            colbias[:, base + 13] = inp['b2'][i]
            for h4 in range(HPC):
                bvrow[i * HPC + h4] = bv_eff[h4 * 128:(h4 + 1) * 128]
        maps.append({
            'xT': _f32(inp['x'][b, :n_tokens].T),
            'encw': _f32(inp['enc_w']),
            'decw': _f32(inp['dec_w']),
            'wqkv': _bf(wqkv),
            'pm': _bf(pmT),
            'wo': _bf(woA),
            'w1': _bf(w1A),
            'w2': _bf(w2A),
            'colbias': colbias,
            'bvrow': _bf(bvrow.reshape(1, -1)),
        })
    return maps


def _qk_bias_zero(inputs):
    inp = {k: np.asarray(v) for k, v in inputs.items()}
    return bool(np.all(inp['bq'] == 0) and np.all(inp['bk'] == 0)
                and np.all(inp['ln1_b'] == 0))


_PROG_CACHE = {}


def _get_program(n_tokens=NTOK, n_cores=8, zero_qkb=True):
    key = (n_tokens, n_cores, zero_qkb)
    if key not in _PROG_CACHE:
        _PROG_CACHE[key] = build_program(n_tokens, n_cores, zero_qkb)
    return _PROG_CACHE[key]


def kernel(**inputs):
    nc = _get_program(zero_qkb=_qk_bias_zero(inputs))
    in_maps = host_prep(inputs)
    res = run_bass_kernel_spmd(nc, in_maps, list(range(8)))
    out = np.stack([res.results[2 * b]['out'][0] for b in range(B)])
    return out.astype(np.float32)


if __name__ == '__main__':
    import pickle
    inp = pickle.load(open('/root/problem/inputs_cache.pkl', 'rb'))
    inp.pop('_ref_jax', None)
    o = kernel(**inp)
    print(o.shape, o.dtype)


# revision 21
# speedup vs baseline: 1.0812x; 1.0812x over previous
"""Trainium2 Bass kernel for nn_CellPerformer (FAVOR+ performer, B=4 N=2048 D=128 H=8 L=4).

Sharding: 8 cores = 4 batches x 2 head-groups (4 heads each). Each core
holds the full residual stream for its batch; attention is head-parallel
within a (batch) pair of cores; the per-layer out-projection partial sums
are AllReduce'd across the pair (chunk-pipelined, overlapped with the
attention epilogue + FFN); LN/FFN are computed redundantly (cheap).

Math plan (validated in numpy vs fp64 reference, relerr ~1.6e-4 from the
structural approximations + ~3e-3 from bf16):
 - KEPS (+1e-4 inside the FAVOR feature) dropped entirely
 - FAVOR max-subtraction dropped (mx=0)
 - q-side diag exp(-|q|^2/2) cancels between numerator and denominator
   (attention output is 0-homogeneous in qf) -> not computed at all
 - k-side diag kept, fused as the ACT-exp per-partition bias
 - LN gamma/beta folded into the following projection weights on the host
 - all big matmuls bf16 with fp32 PSUM accumulation; enc/dec fp32
 - gelu = tanh approximation (matches jax.nn.gelu default)
 - v-bias handled exactly via a rank-1 correction on ctx

Scheduling: ACT table loads minimized by clustering Ln ops (Square/Copy/
Identity/Exp co-reside in one table set; Ln and Gelu force switches, so
they are batched 4-wide).  Next layer's LN1 + q/k/v projections are
emitted in the current layer's epilogue so the PE stays busy during the
ACT-heavy LN/gelu stretch.  PSUM: psK 2x[128,1024] (kf px double-buffer /
merged qfe pairs / FFN pf), psC 1x[128,640] (pctx / potq / pf2), psA
2x[128,512] (proj, pa, rows) + 4x[128,512] (LN stats).
"""
import math
import os
import sys

import numpy as np
import ml_dtypes

for _p in ('/opt/trn_rl_repo', '/root/.axon_site/_ro/trn_rl_repo'):
    if os.path.isdir(_p) and _p not in sys.path:
        sys.path.append(_p)

import concourse.bass as bass
import concourse.tile as tile
from concourse import bacc, mybir
from concourse.bass_utils import run_bass_kernel_spmd
from concourse.masks import make_identity

FP = mybir.dt.float32
BF = mybir.dt.bfloat16
AF = mybir.ActivationFunctionType
ALU = mybir.AluOpType

B, NTOK, M = 4, 2048, 35
D, H, L = 128, 8, 4
FD = 621
FF = 512
HPC = 4                      # heads per core
NORM = D ** -0.25
FC = [128, 128, 128, 128, 109]   # FD chunk sizes
NCH = 5
CB_PER_LAYER = 14            # colbias columns per layer
GELU_AF = None               # resolved at build: AF.Gelu_apprx_tanh


def build_program(n_tokens=NTOK, n_cores=8, zero_qkb=True):
    NT = n_tokens // 128      # 128-token tiles
    NQ = n_tokens // 512      # 512-token chunks
    groups = [[2 * i, 2 * i + 1] for i in range(n_cores // 2)]

    nc = bacc.Bacc("TRN2", target_bir_lowering=False, debug=False,
                   num_devices=n_cores)

    xT = nc.dram_tensor("xT", [M, n_tokens], FP, kind="ExternalInput")
    encw = nc.dram_tensor("encw", [M, D], FP, kind="ExternalInput")
    decw = nc.dram_tensor("decw", [D, 1], FP, kind="ExternalInput")
    wqkv = nc.dram_tensor("wqkv", [L, 3, D, 512], BF, kind="ExternalInput")
    pm = nc.dram_tensor("pm", [L, D, FD], BF, kind="ExternalInput")
    wo = nc.dram_tensor("wo", [L, D, HPC * D], BF, kind="ExternalInput")
    w1 = nc.dram_tensor("w1", [L, D, FF], BF, kind="ExternalInput")
    w2 = nc.dram_tensor("w2", [L, FF // 4, 4 * D], BF, kind="ExternalInput")
    colbias = nc.dram_tensor("colbias", [D, 2 + CB_PER_LAYER * L], FP,
                             kind="ExternalInput")
    bvrow = nc.dram_tensor("bvrow", [1, HPC * L * D], BF, kind="ExternalInput")
    out = nc.dram_tensor("out", [1, n_tokens], FP, kind="ExternalOutput")

    from contextlib import ExitStack
    with tile.TileContext(nc) as tc:
        with ExitStack() as st:
            _emit(st, tc, nc, locals(), NT, NQ, n_tokens, groups, zero_qkb)

    nc.compile()
    return nc


def _emit(st, tc, nc, IO, NT, NQ, n_tokens, groups, zero_qkb):
    xT, encw, decw = IO['xT'], IO['encw'], IO['decw']
    wqkv, pm, wo, w1, w2 = IO['wqkv'], IO['pm'], IO['wo'], IO['w1'], IO['w2']
    colbias_d, bvrow_d, out_d = IO['colbias'], IO['bvrow'], IO['out']

    consts = st.enter_context(tc.tile_pool(name="consts", bufs=1))
    wts = st.enter_context(tc.tile_pool(name="wts", bufs=2))
    acts = st.enter_context(tc.tile_pool(name="acts", bufs=1))
    sm = st.enter_context(tc.tile_pool(name="sm", bufs=2))
    psK = st.enter_context(tc.tile_pool(name="psK", bufs=2, space="PSUM"))
    psC = st.enter_context(tc.tile_pool(name="psC", bufs=1, space="PSUM"))
    psA = st.enter_context(tc.tile_pool(name="psA", bufs=2, space="PSUM"))
    dram = st.enter_context(tc.tile_pool(name="dram", bufs=8, space="DRAM"))

    # ---- constants ----
    ones_bf = consts.tile([128, 1], BF, tag="ones_bf")
    nc.vector.memset(ones_bf, 1.0)
    onesc = consts.tile([128, 128], FP, tag="onesc")
    nc.vector.memset(onesc, 1.0 / 128.0)
    onesc_bf = consts.tile([128, 128], BF, tag="onesc_bf")
    nc.vector.memset(onesc_bf, 1.0 / 128.0)
    halfneg = consts.tile([128, 1], BF, tag="halfneg")
    nc.vector.memset(halfneg, -0.5)
    ident_bf = consts.tile([128, 128], BF, tag="ident_bf")
    make_identity(nc, ident_bf)
    eps_col = consts.tile([128, 1], FP, tag="eps_col")
    nc.vector.memset(eps_col, 1e-5)
    cb = consts.tile([128, 2 + CB_PER_LAYER * L], FP, tag="cb")
    nc.sync.dma_start(out=cb, in_=colbias_d[:, :])
    bvr = consts.tile([1, HPC * L * D], BF, tag="bvr")
    nc.sync.dma_start(out=bvr, in_=bvrow_d[:, :])
    encw_t = consts.tile([M, 128], FP, tag="encw_t")
    nc.sync.dma_start(out=encw_t, in_=encw[:, :])
    decw_t = consts.tile([128, 1], FP, tag="decw_t")
    nc.sync.dma_start(out=decw_t, in_=decw[:, :])
    xT_t = consts.tile([M, n_tokens], FP, tag="xT_t")
    nc.sync.dma_start(out=xT_t, in_=xT[:, :])

    y = acts.tile([128, n_tokens], FP, tag="y")

    # ---- encoder (fp32): y^T = encw^T @ x^T + enc_b ----
    for tq in range(NQ):
        p = psA.tile([128, 512], FP, tag="u")
        nc.tensor.matmul(p, encw_t, xT_t[:, tq * 512:(tq + 1) * 512],
                         start=True, stop=True)
        nc.vector.tensor_scalar_add(y[:, tq * 512:(tq + 1) * 512], p, cb[:, 0:1])

    def emit_ln(dst):
        """dst = (y - mu)*rsqrt(var+eps).  Ln and Exp ops batched 4-wide so
        the ACT table set switches at most twice.  ymu is computed eagerly
        so the PSUM stat tiles are released before the batched Ln phase."""
        ymus, work = [], []
        for tq in range(NQ):
            sl = slice(tq * 512, (tq + 1) * 512)
            ysq = sm.tile([128, 512], BF, tag="ysq", bufs=2)
            nc.vector.tensor_mul(ysq, y[:, sl], y[:, sl])
            pS = psA.tile([128, 512], FP, tag="u")
            nc.tensor.matmul(pS, onesc, y[:, sl], start=True, stop=True)
            pSS = psA.tile([128, 512], FP, tag="u")
            nc.tensor.matmul(pSS, onesc_bf, ysq, start=True, stop=True)
            musq = sm.tile([128, 512], FP, tag="musq", bufs=1)
            nc.scalar.activation(musq, pS, AF.Square)
            ymu = sm.tile([128, 512], BF, tag="ymu", bufs=4)
            nc.vector.tensor_sub(ymu, y[:, sl], pS)
            varr = sm.tile([128, 512], FP, tag="varr", bufs=4)
            nc.vector.tensor_sub(varr, pSS, musq)
            ymus.append(ymu)
            work.append(varr)
        for tq in range(NQ):
            lnv = sm.tile([128, 512], FP, tag="lnv", bufs=4)
            nc.scalar.activation(lnv, work[tq], AF.Ln, bias=eps_col[:, 0:1])
            work[tq] = lnv
        for tq in range(NQ):
            rs = sm.tile([128, 512], BF, tag="rs", bufs=4)
            nc.scalar.activation(rs, work[tq], AF.Exp, scale=-0.5)
            work[tq] = rs
        for tq in range(NQ):
            sl = slice(tq * 512, (tq + 1) * 512)
            nc.vector.tensor_mul(dst[:, sl], ymus[tq], work[tq])

    def emit_proj(li, h, qh, kh, vK, ndk):
        """q/k/v projections for all heads + k-diag columns."""
        base = 2 + CB_PER_LAYER * li
        wqkv_t = wts.tile([128, 3, 512], BF, tag="wqkv_t", name="wqkv_t")
        nc.sync.dma_start(out=wqkv_t, in_=wqkv[li].rearrange("t d f -> d t f"))
        for h4 in range(HPC):
            wq_c = wqkv_t[:, 0, h4 * 128:(h4 + 1) * 128]
            wk_c = wqkv_t[:, 1, h4 * 128:(h4 + 1) * 128]
            for tq in range(NQ):
                sl = slice(tq * 512, (tq + 1) * 512)
                pq = psA.tile([128, 512], FP, tag="u")
                nc.tensor.matmul(pq, wq_c, h[:, sl], start=True, stop=True)
                if zero_qkb:
                    nc.vector.tensor_copy(qh[h4][:, sl], pq)
                else:
                    nc.vector.tensor_scalar_add(
                        qh[h4][:, sl], pq, cb[:, base + h4:base + h4 + 1])
                pk = psA.tile([128, 512], FP, tag="u")
                nc.tensor.matmul(pk, wk_c, h[:, sl], start=True, stop=True)
                if zero_qkb:
                    nc.vector.tensor_copy(kh[h4][:, sl], pk)
                else:
                    nc.vector.tensor_scalar_add(
                        kh[h4][:, sl], pk, cb[:, base + 4 + h4:base + 5 + h4])
        for tt in range(NT):
            pv = psK.tile([128, 1024], FP, tag="px")
            nc.tensor.matmul(pv[:, 0:512], h[:, tt * 128:(tt + 1) * 128],
                             wqkv_t[:, 2, :], start=True, stop=True)
            nc.vector.tensor_copy(vK[:, tt * 512:(tt + 1) * 512], pv[:, 0:512])
        # k-diag columns: ndk[tok, h4*16+tt] = -0.5 * sum_d k^2
        pkc = psA.tile([128, 64], FP, tag="u")
        ksqs = []
        for h4 in range(HPC):
            ksq = sm.tile([128, n_tokens], BF, tag="ksq", bufs=2, name="ksq")
            nc.vector.tensor_mul(ksq, kh[h4], kh[h4])
            ksqs.append(ksq)
        for h4 in range(HPC):
            for tt in range(NT):
                nc.tensor.matmul(pkc[:, h4 * 16 + tt:h4 * 16 + tt + 1],
                                 ksqs[h4][:, tt * 128:(tt + 1) * 128], halfneg,
                                 start=(h4 == 0 and tt == 0),
                                 stop=(h4 == HPC - 1 and tt == NT - 1))
        nc.vector.tensor_copy(ndk, pkc)

    # ---- layer-0 prologue ----
    h = acts.tile([128, n_tokens], BF, tag="h", bufs=2)
    emit_ln(h)
    qh = [acts.tile([128, n_tokens], BF, tag=f"qh{h4}", name=f"qh{h4}", bufs=1)
          for h4 in range(HPC)]
    kh = [acts.tile([128, n_tokens], BF, tag=f"kh{h4}", name=f"kh{h4}", bufs=1)
          for h4 in range(HPC)]
    vK = acts.tile([128, NT * 512], BF, tag="vK", bufs=1)
    ndk = sm.tile([128, 64], FP, tag="ndk", bufs=2)
    emit_proj(0, h, qh, kh, vK, ndk)

    for li in range(L):
        base = 2 + CB_PER_LAYER * li
        # ---- stage attention/ffn weights ----
        pm_t = wts.tile([128, FD], BF, tag="pm_t")
        nc.sync.dma_start(out=pm_t, in_=pm[li])
        wo_t = wts.tile([128, HPC * 128], BF, tag="wo_t")
        nc.sync.dma_start(out=wo_t, in_=wo[li])
        w1_t = wts.tile([128, FF], BF, tag="w1_t")
        nc.sync.dma_start(out=w1_t, in_=w1[li])
        w2_t = wts.tile([128, 4 * 128], BF, tag="w2_t")
        nc.sync.dma_start(out=w2_t, in_=w2[li])

        # ---- per-head: kf features, ctx, ksum, Raug ----
        Raugs = []
        for h4 in range(HPC):
            pctx = psC.tile([128, 640], FP, tag="ctx")
            pksa = psA.tile([1, 512], FP, tag="u")
            pksb = psA.tile([1, 128], FP, tag="u")
            for tt in range(NT):
                px = psK.tile([128, 1024], FP, tag="px")
                kh_t = kh[h4][:, tt * 128:(tt + 1) * 128]
                nc.tensor.matmul(px[:, 0:512], kh_t, pm_t[:, 0:512],
                                 start=True, stop=True)
                nc.tensor.matmul(px[:, 512:FD], kh_t, pm_t[:, 512:FD],
                                 start=True, stop=True)
                kf = acts.tile([128, FD], BF, tag="kf", bufs=6, name="kf")
                nc.scalar.activation(kf, px[:, 0:FD], AF.Exp,
                                     bias=ndk[:, h4 * 16 + tt:h4 * 16 + tt + 1])
                vk_t = vK[:, tt * 512 + h4 * 128:tt * 512 + (h4 + 1) * 128]
                nc.tensor.matmul(pctx[:, 0:512], vk_t, kf[:, 0:512],
                                 start=(tt == 0), stop=False)
                nc.tensor.matmul(pctx[:, 512:FD], vk_t, kf[:, 512:FD],
                                 start=(tt == 0), stop=False)
                nc.tensor.matmul(pksa, ones_bf, kf[:, 0:512],
                                 start=(tt == 0), stop=(tt == NT - 1))
                nc.tensor.matmul(pksb[0:1, 0:FD - 512], ones_bf, kf[:, 512:FD],
                                 start=(tt == 0), stop=(tt == NT - 1))
            ks_row = sm.tile([1, FD], BF, tag="ks_row", bufs=2)
            nc.vector.tensor_copy(ks_row[0:1, 0:512], pksa)
            nc.vector.tensor_copy(ks_row[0:1, 512:FD], pksb[0:1, 0:FD - 512])
            # v-bias rank-1 correction: ctx += bv (x) ksum
            rb = bvr[0:1, (li * HPC + h4) * 128:(li * HPC + h4 + 1) * 128]
            nc.tensor.matmul(pctx[:, 0:512], rb, ks_row[0:1, 0:512],
                             start=False, stop=True)
            nc.tensor.matmul(pctx[:, 512:FD], rb, ks_row[0:1, 512:FD],
                             start=False, stop=True)
            ctxT = sm.tile([128, FD], BF, tag="ctxT", bufs=2)
            nc.vector.tensor_copy(ctxT, pctx[:, 0:FD])
            # ksum -> columns [FC, 1] per chunk (rank-1 matmuls into one
            # bank; a final full-partition dummy write closes the group)
            pst = psA.tile([128, 8], FP, tag="u")
            nc.vector.memset(pst, 0.0)
            for c in range(NCH):
                nc.tensor.matmul(pst[0:FC[c], c:c + 1],
                                 ks_row[0:1, c * 128:c * 128 + FC[c]],
                                 ones_bf[0:1, 0:1],
                                 start=(c == 0), stop=False)
            nc.tensor.matmul(pst[:, 5:6], ks_row[0:1, 0:128],
                             ones_bf[0:1, 0:1], start=False, stop=True)
            kscol = sm.tile([128, 8], BF, tag="kscol", bufs=2)
            nc.vector.tensor_copy(kscol, pst)

            # Raug chunks: [ctx^T chunk @ wo | ksum chunk]
            Raug = sm.tile([128, NCH * 129], BF, tag=f"Raug{h4}", bufs=1,
                           name=f"Raug{h4}")
            for c in range(NCH):
                pcw = psA.tile([128, 512], FP, tag="u")
                nc.tensor.matmul(pcw[0:FC[c], 0:128],
                                 ctxT[:, c * 128:c * 128 + FC[c]],
                                 wo_t[:, h4 * 128:(h4 + 1) * 128],
                                 start=True, stop=True)
                nc.vector.tensor_copy(Raug[0:FC[c], c * 129:c * 129 + 128],
                                      pcw[0:FC[c], 0:128])
            nc.vector.tensor_copy(
                Raug.rearrange("p (c w) -> p c w", w=129)[:, :, 128],
                kscol[:, 0:NCH])
            Raugs.append(Raug)

        # ---- pa phase: per 512-chunk: qfe gen (pair-merged exps), pa,
        #      transpose-accumulate into o^T PSUM, wire out, AllReduce.
        #      qfe generation runs one chunk ahead of pa so the PE never
        #      waits on the ACT exp stream. ----
        ccouts = [None] * NQ

        def emit_qfe_gen(tq):
            sl = slice(tq * 512, (tq + 1) * 512)
            qfes = []
            for h4 in range(HPC):
                qfe = sm.tile([128, NCH * 512], BF, tag=f"qfe{h4}", bufs=2,
                              name=f"qfe{h4}")
                for cp in range(3):      # chunk pairs (0,1), (2,3), (4,)
                    pqf = psK.tile([128, 1024], FP, tag="px")
                    w = 0
                    for c in range(2 * cp, min(2 * cp + 2, NCH)):
                        nc.tensor.matmul(pqf[0:FC[c], w:w + 512],
                                         pm_t[:, c * 128:c * 128 + FC[c]],
                                         qh[h4][:, sl], start=True, stop=True)
                        w += 512
                    pr = 128 if cp < 2 else FC[4]
                    nc.scalar.activation(
                        qfe[0:pr, 2 * cp * 512:2 * cp * 512 + w],
                        pqf[0:pr, 0:w], AF.Exp)
                qfes.append(qfe)
            return qfes

        def emit_pa_body(tq, qfes):
            potq = psC.tile([128, 640], FP, tag="ctx")
            for t4 in range(4):
                ots = []
                for h4 in range(HPC):
                    pa = psA.tile([128, 512], FP, tag="u")
                    for c in range(NCH):
                        nc.tensor.matmul(
                            pa[:, 0:129],
                            qfes[h4][0:FC[c], c * 512 + t4 * 128:
                                     c * 512 + (t4 + 1) * 128],
                            Raugs[h4][0:FC[c], c * 129:(c + 1) * 129],
                            start=(c == 0), stop=(c == NCH - 1))
                    sc = sm.tile([128, 1], FP, tag="sc", bufs=4)
                    nc.vector.reciprocal(sc, pa[:, 128:129])
                    ot = sm.tile([128, 128], BF, tag="ot", bufs=8)
                    nc.vector.tensor_scalar_mul(ot, pa[:, 0:128], sc)
                    ots.append(ot)
                for h4 in range(HPC):
                    nc.tensor.matmul(potq[:, t4 * 128:(t4 + 1) * 128],
                                     ots[h4], ident_bf,
                                     start=(t4 == 0 and h4 == 0),
                                     stop=(t4 == 3 and h4 == HPC - 1))
            wire = sm.tile([128, 512], BF, tag="wire", bufs=2)
            nc.scalar.activation(wire, potq[:, 0:512], AF.Identity,
                                 bias=cb[:, base + 12:base + 13])
            ccin = dram.tile([128, 512], BF, tag="ccin", bufs=8)
            ccout = dram.tile([128, 512], BF, tag="ccout", bufs=8)
            nc.sync.dma_start(out=ccin, in_=wire)
            nc.gpsimd.collective_compute(
                "AllReduce", ALU.add, replica_groups=groups,
                ins=[ccin.opt()], outs=[ccout.opt()])
            ccouts[tq] = ccout

        qprev = emit_qfe_gen(0)
        for tq in range(1, NQ):
            qnext = emit_qfe_gen(tq)
            emit_pa_body(tq - 1, qprev)
            qprev = qnext
        emit_pa_body(NQ - 1, qprev)

        # ---- epilogue: y += AR result, LN2 (Ln-batched), FFN; then next
        #      layer's LN1 + projections to keep the PE fed ----
        for tq in range(NQ):
            sl = slice(tq * 512, (tq + 1) * 512)
            asum = sm.tile([128, 512], BF, tag="asum", bufs=2)
            nc.sync.dma_start(out=asum, in_=ccouts[tq])
            nc.vector.tensor_add(y[:, sl], y[:, sl], asum)
        h2 = acts.tile([128, n_tokens], BF, tag="h", bufs=2, name="h2")
        emit_ln(h2)
        for tq in range(NQ):
            sl = slice(tq * 512, (tq + 1) * 512)
            gl = sm.tile([128, 4 * 512], BF, tag="gl", bufs=1)
            for c in range(4):
                pf = psK.tile([128, 1024], FP, tag="px")
                nc.tensor.matmul(pf[:, 0:512], w1_t[:, c * 128:(c + 1) * 128],
                                 h2[:, sl], start=True, stop=True)
                nc.scalar.activation(gl[:, c * 512:(c + 1) * 512],
                                     pf[:, 0:512], GELU_AF or AF.Gelu_apprx_tanh,
                                     bias=cb[:, base + 8 + c:base + 9 + c])
            pf2 = psC.tile([128, 640], FP, tag="ctx")
            for c in range(4):
                nc.tensor.matmul(pf2[:, 0:512], w2_t[:, c * 128:(c + 1) * 128],
                                 gl[:, c * 512:(c + 1) * 512],
                                 start=(c == 0), stop=(c == 3))
            ffnd = sm.tile([128, 512], BF, tag="ffnd", bufs=2)
            nc.scalar.activation(ffnd, pf2[:, 0:512], AF.Identity,
                                 bias=cb[:, base + 13:base + 14])
            nc.vector.tensor_add(y[:, sl], y[:, sl], ffnd)
        if li < L - 1:
            h = acts.tile([128, n_tokens], BF, tag="h", bufs=2, name="h")
            emit_ln(h)
            qh = [acts.tile([128, n_tokens], BF, tag=f"qh{h4}",
                            name=f"qh{h4}", bufs=1) for h4 in range(HPC)]
            kh = [acts.tile([128, n_tokens], BF, tag=f"kh{h4}",
                            name=f"kh{h4}", bufs=1) for h4 in range(HPC)]
            vK = acts.tile([128, NT * 512], BF, tag="vK", bufs=1)
            ndk = sm.tile([128, 64], FP, tag="ndk", bufs=2)
            emit_proj(li + 1, h, qh, kh, vK, ndk)

    # ---- decoder (fp32) ----
    for tq in range(NQ):
        pd = psA.tile([1, 512], FP, tag="u")
        nc.tensor.matmul(pd, decw_t, y[:, tq * 512:(tq + 1) * 512],
                         start=True, stop=True)
        orow = sm.tile([1, 512], FP, tag="orow", bufs=2)
        nc.vector.tensor_scalar_add(orow, pd, cb[0:1, 1:2])
        nc.sync.dma_start(out=out_d[0:1, tq * 512:(tq + 1) * 512], in_=orow)


# --------------------------------------------------------------------------
# host side
# --------------------------------------------------------------------------

def _bf(x):
    return np.ascontiguousarray(x).astype(ml_dtypes.bfloat16)


def _f32(x):
    return np.ascontiguousarray(x, dtype=np.float32)


def host_prep(inputs, n_tokens=NTOK, n_cores=8):
    """Full inputs -> per-core input dicts."""
    inp = {k: np.asarray(v, dtype=np.float32) for k, v in inputs.items()}
    maps = []
    for core in range(n_cores):
        b = core // 2
        hg = core % 2
        hsl = slice(hg * HPC * D, (hg + 1) * HPC * D)   # 512 head cols
        C = 2 + CB_PER_LAYER * L
        colbias = np.zeros((D, C), np.float32)
        colbias[:, 0] = inp['enc_b']
        colbias[0, 1] = inp['dec_b'][0]
        wqkv = np.zeros((L, 3, D, 512), np.float32)
        pmT = np.zeros((L, D, FD), np.float32)
        woA = np.zeros((L, D, HPC * D), np.float32)
        w1A = np.zeros((L, D, FF), np.float32)
        w2A = np.zeros((L, FF // 4, 4 * D), np.float32)
        bvrow = np.zeros((HPC * L, D), np.float32)
        for i in range(L):
            g1, b1 = inp['ln1_g'][i], inp['ln1_b'][i]
            g2, b2v = inp['ln2_g'][i], inp['ln2_b'][i]
            base = 2 + CB_PER_LAYER * i
            wqkv[i, 0] = g1[:, None] * inp['wq'][i][:, hsl] * NORM
            wqkv[i, 1] = g1[:, None] * inp['wk'][i][:, hsl] * NORM
            wqkv[i, 2] = g1[:, None] * inp['wv'][i][:, hsl]
            bq_eff = (b1 @ inp['wq'][i][:, hsl] + inp['bq'][i][hsl]) * NORM
            bk_eff = (b1 @ inp['wk'][i][:, hsl] + inp['bk'][i][hsl]) * NORM
            bv_eff = b1 @ inp['wv'][i][:, hsl] + inp['bv'][i][hsl]
            colbias[:, base:base + 4] = bq_eff.reshape(4, 128).T
            colbias[:, base + 4:base + 8] = bk_eff.reshape(4, 128).T
            pmT[i] = inp['proj'][i].T
            wo_sl = inp['wo'][i][hsl, :].reshape(HPC, D, D)      # [h, d, m]
            woA[i] = wo_sl.transpose(1, 0, 2).reshape(D, HPC * D)
            w1A[i] = g2[:, None] * inp['w1'][i]
            b1_eff = b2v @ inp['w1'][i] + inp['b1'][i]
            colbias[:, base + 8:base + 12] = b1_eff.reshape(4, 128).T
            w2c = inp['w2'][i].reshape(4, 128, D)
            w2A[i] = w2c.transpose(1, 0, 2).reshape(128, 4 * D)
            # bo halved: the pair AllReduce sums it from both cores
            colbias[:, base + 12] = 0.5 * inp['bo'][i]
            colbias[:, base + 13] = inp['b2'][i]
            for h4 in range(HPC):
                bvrow[i * HPC + h4] = bv_eff[h4 * 128:(h4 + 1) * 128]
        maps.append({
            'xT': _f32(inp['x'][b, :n_tokens].T),
            'encw': _f32(inp['enc_w']),
            'decw': _f32(inp['dec_w']),
            'wqkv': _bf(wqkv),
            'pm': _bf(pmT),
            'wo': _bf(woA),
            'w1': _bf(w1A),
            'w2': _bf(w2A),
            'colbias': colbias,
            'bvrow': _bf(bvrow.reshape(1, -1)),
        })
    return maps


def _qk_bias_zero(inputs):
    inp = {k: np.asarray(v) for k, v in inputs.items()}
    return bool(np.all(inp['bq'] == 0) and np.all(inp['bk'] == 0)
                and np.all(inp['ln1_b'] == 0))


_PROG_CACHE = {}


def _get_program(n_tokens=NTOK, n_cores=8, zero_qkb=True):
    key = (n_tokens, n_cores, zero_qkb)
    if key not in _PROG_CACHE:
        _PROG_CACHE[key] = build_program(n_tokens, n_cores, zero_qkb)
    return _PROG_CACHE[key]


def kernel(**inputs):
    nc = _get_program(zero_qkb=_qk_bias_zero(inputs))
    in_maps = host_prep(inputs)
    res = run_bass_kernel_spmd(nc, in_maps, list(range(8)))
    out = np.stack([res.results[2 * b]['out'][0] for b in range(B)])
    return out.astype(np.float32)


if __name__ == '__main__':
    import pickle
    inp = pickle.load(open('/root/problem/inputs_cache.pkl', 'rb'))
    inp.pop('_ref_jax', None)
    o = kernel(**inp)
    print(o.shape, o.dtype)
